# revision 43
# baseline (speedup 1.0000x reference)
"""LIPAR segment attention kernel for TRN2, 8 NeuronCores.

Problem (hardcoded): B=4, N=4096, DIM=768, H=12 heads, DH=64, S=16 segments
of M=256 tokens. q = x@Wq (scaled, rotary), kv = x@Wkv (rotary, shared K==V).
Segment t>=1 attends to segments [t-1, t]; segment 0 attends to itself.
Output projection Wo/bo for segments >=1, Wo0/bo0 for segment 0.

Sharding: the (b h) = 48 fused axis is split 8 ways -> 6 heads per core;
core i -> batch i//2, heads (i%2)*6 .. (i%2)*6+6. Each core computes a
partial output projection (its 384 feature rows of Wo/Wo0); the host sums
the two partial (768, 4096) results per batch and adds biases.

On-device strategy (bf16 matmul pipeline, fp32 PSUM accumulation):
  - all-bf16 inputs/weights/tables -> every matmul at 1 cycle/row.
  - software pipelining: block b's projections+rotary ("produce") are
    emitted interleaved with block b-1's attention ("attend") at t-group
    granularity so every engine always has independent queued work
    (engine streams execute strictly in order).
  - rotary in the transposed layout: rot = raw*cosT + (Pshift@raw)*sinT,
    sign baked into sinT. raw copies on ACT, sin-mul on DVE (PSUM read),
    cos-mul + add on Pool.
  - QK^T computes S^T (keys-on-partition); exp on ACT (no max subtraction),
    bf16 out.
  - AV in natural orientation (out(query, dh) = pt_chunk^T @ kvn with a
    ones column for the denominator): softmax denom is per-partition, so
    normalization is a DVE reciprocal + two per-partition tensor_scalar
    multiplies straight out of PSUM.
  - normalized (query, feat-pair) tiles are PE-transposed back to
    (feat, token); 4 transposes share one PSUM bank and are copied off
    with a single wide DVE copy (same for the kv natural-layout build).
  - output projection consumes outT directly; partial (768, 4096) fp32
    results are DMA'd out; host sums core pairs, transposes, adds biases.
  - startup DMA is need-ordered (first projection's weight chunk, block-0
    inputs, shift matrix, then the rest) so the PE starts within ~2us.

Measured (cost-model timeline, per core): 210011 ns vs 844046 ns for the
fp32 non-pipelined baseline (4.02x). HW rel l2 error 5.3e-3 (gate 2e-2).
The final block's attention is pipelined one t-section behind its produce
phase (instead of a full block) so the epilogue is a single section.
"""

import numpy as np

B, N, DIM = 4, 4096, 768
H = 12
DH = 64
S = 16
M = 256
SCALE = DH**-0.5

HPC = 6            # heads per core
FPC = HPC * DH     # 384 features per core
KC = DIM // 128    # 6 contraction chunks
NB = 8             # token blocks
TB = N // NB       # 512 tokens per block (2 segments)
NCORES = 8


def _host_tables():
    """cosT/sinT (128, N) in the 2-head-stacked transposed layout and
    the signed shift permutation (128, 128)."""
    inv_freq = 1.0 / (10000.0 ** (np.arange(0, DH, 2, dtype=np.float64) / DH))
    t = np.arange(N, dtype=np.float64)
    freqs = np.outer(inv_freq, t)            # (32, N)
    r = np.arange(128)
    fidx = r % 32
    cosT = np.cos(freqs)[fidx].astype(np.float32)            # (128, N)
    sign = np.where((r % 64) < 32, -1.0, 1.0)[:, None]
    sinT = (sign * np.sin(freqs)[fidx]).astype(np.float32)   # (128, N)
    pshift = np.zeros((128, 128), dtype=np.float32)
    for m_ in range(128):
        src = m_ + 32 if (m_ % 64) < 32 else m_ - 32
        pshift[src, m_] = 1.0
    return cosT, sinT, pshift


def _build_nc():
    import concourse.bass as bass
    import concourse.bacc as bacc
    import concourse.tile as tile
    from concourse import mybir
    from concourse.masks import make_identity
    from contextlib import ExitStack

    f32 = mybir.dt.float32
    bf16 = mybir.dt.bfloat16
    EXP = mybir.ActivationFunctionType.Exp
    MULT = mybir.AluOpType.mult

    nc = bacc.Bacc("TRN2", target_bir_lowering=False)
    xT = nc.dram_tensor("xT", [DIM, N], bf16, kind="ExternalInput")
    wq = nc.dram_tensor("wq", [DIM, FPC], bf16, kind="ExternalInput")
    wkv = nc.dram_tensor("wkv", [DIM, FPC], bf16, kind="ExternalInput")
    wo = nc.dram_tensor("wo", [FPC, DIM], bf16, kind="ExternalInput")
    wo0 = nc.dram_tensor("wo0", [FPC, DIM], bf16, kind="ExternalInput")
    cosT = nc.dram_tensor("cosT", [128, N], bf16, kind="ExternalInput")
    sinT = nc.dram_tensor("sinT", [128, N], bf16, kind="ExternalInput")
    pshift = nc.dram_tensor("pshift", [128, 128], bf16, kind="ExternalInput")
    outpT = nc.dram_tensor("outpT", [DIM, N], f32, kind="ExternalOutput")

    outpT_r = outpT.rearrange("(c p) n -> p c n", p=128)

    with tile.TileContext(nc) as tc, ExitStack() as ctx:
        consts = ctx.enter_context(tc.tile_pool(name="consts", bufs=1))
        xpool = ctx.enter_context(tc.tile_pool(name="xpool", bufs=2))
        cspool = ctx.enter_context(tc.tile_pool(name="cspool", bufs=2))
        rawpool = ctx.enter_context(tc.tile_pool(name="rawpool", bufs=3))
        tmppool = ctx.enter_context(tc.tile_pool(name="tmppool", bufs=4))
        qrpool = ctx.enter_context(tc.tile_pool(name="qrpool", bufs=2))
        kvrpool = ctx.enter_context(tc.tile_pool(name="kvrpool", bufs=3))
        kvnpool = ctx.enter_context(tc.tile_pool(name="kvnpool", bufs=3))
        ptpool = ctx.enter_context(tc.tile_pool(name="ptpool", bufs=16))
        otpool = ctx.enter_context(tc.tile_pool(name="otpool", bufs=3))
        prpool = ctx.enter_context(tc.tile_pool(name="prpool", bufs=3))
        natpool = ctx.enter_context(tc.tile_pool(name="natp", bufs=8))
        rcpool = ctx.enter_context(tc.tile_pool(name="rcp", bufs=8))

        mmps = ctx.enter_context(tc.tile_pool(name="mmps", bufs=2, space="PSUM"))
        tpps = ctx.enter_context(tc.tile_pool(name="tpps", bufs=1, space="PSUM"))
        stps = ctx.enter_context(tc.tile_pool(name="stps", bufs=3, space="PSUM"))
        avps = ctx.enter_context(tc.tile_pool(name="avps", bufs=2, space="PSUM"))

        xT_r = xT.rearrange("(c p) n -> p c n", p=128)

        # per-block live state (rotary outputs, kv natural, outT)
        st_ = {}

        def load_block(bl):
            n0 = bl * TB
            xt = xpool.tile([128, KC, TB], bf16, tag="xt")
            nc.sync.dma_start(out=xt[:, 0:3, :], in_=xT_r[:, 0:3, n0 : n0 + TB])
            nc.sync.dma_start(out=xt[:, 3:KC, :], in_=xT_r[:, 3:KC, n0 : n0 + TB])
            cosb = cspool.tile([128, TB], bf16, tag="cosb")
            nc.sync.dma_start(out=cosb, in_=cosT[:, n0 : n0 + TB])
            sinb = cspool.tile([128, TB], bf16, tag="sinb")
            nc.sync.dma_start(out=sinb, in_=sinT[:, n0 : n0 + TB])
            st_[(bl, "in")] = (xt, cosb, sinb)

        # resident constants; block-0 inputs first so the first projection
        # is not stuck behind the full weight preload
        ident = consts.tile([128, 128], bf16, tag="ident")
        make_identity(nc, ident)
        wq_r = wq.rearrange("(c p) m -> p c m", p=128)
        wkv_r = wkv.rearrange("(c p) m -> p c m", p=128)
        wq_sb = consts.tile([128, KC, FPC], bf16, tag="wq_sb")
        wkv_sb = consts.tile([128, KC, FPC], bf16, tag="wkv_sb")
        psh_sb = consts.tile([128, 128], bf16, tag="psh_sb")
        onesb = consts.tile([128, 1], bf16, tag="onesb")
        nc.vector.memset(onesb, 1.0)
        # need-ordered preload: first projection's weights, block-0 inputs,
        # the shift matrix, then the rest
        nc.sync.dma_start(out=wq_sb[:, 0:1, 0:128], in_=wq_r[:, 0:1, 0:128])
        nc.sync.dma_start(out=wq_sb[:, 1:KC, 0:128], in_=wq_r[:, 1:KC, 0:128])
        load_block(0)
        nc.sync.dma_start(out=psh_sb, in_=pshift[:, :])
        nc.sync.dma_start(out=wkv_sb[:, :, 0:128], in_=wkv_r[:, :, 0:128])
        nc.sync.dma_start(out=wq_sb[:, :, 128:FPC], in_=wq_r[:, :, 128:FPC])
        nc.sync.dma_start(out=wkv_sb[:, :, 128:FPC], in_=wkv_r[:, :, 128:FPC])
        wo_sb = consts.tile([128, 3, DIM], bf16, tag="wo_sb")
        nc.sync.dma_start(out=wo_sb, in_=wo.rearrange("(c p) m -> p c m", p=128))
        wo0_sb = consts.tile([128, 3, DIM], bf16, tag="wo0_sb")
        nc.sync.dma_start(out=wo0_sb, in_=wo0.rearrange("(c p) m -> p c m", p=128))

        def proj(wsb, xt, dst_rot_raw, t, on_dve=False):
            """projection matmuls for one 128-feature t-group + raw copy."""
            raw = dst_rot_raw
            ps = mmps.tile([128, TB], f32, tag="mmps")
            for c in range(KC):
                nc.tensor.matmul(
                    ps,
                    lhsT=wsb[:, c, t * 128 : (t + 1) * 128],
                    rhs=xt[:, c, :],
                    start=(c == 0),
                    stop=(c == KC - 1),
                )
            if on_dve:
                nc.vector.tensor_copy(raw[:, t, :], ps)
            else:
                nc.scalar.copy(raw[:, t, :], ps)

        def rot_shift(raw, t):
            shps = mmps.tile([128, TB], f32, tag="mmps")
            nc.tensor.matmul(
                shps, lhsT=psh_sb, rhs=raw[:, t, :], start=True, stop=True
            )
            return shps

        def rot_muls(raw, shps, t, cosb, sinb):
            tmp2 = tmppool.tile([128, TB], bf16, tag="tmp2")
            nc.vector.tensor_mul(tmp2, shps, sinb)
            tmp1 = tmppool.tile([128, TB], bf16, tag="tmp1")
            nc.gpsimd.tensor_mul(tmp1, raw[:, t, :], cosb)
            return tmp1, tmp2

        def rot_add(rot, t, tmp1, tmp2):
            nc.gpsimd.tensor_add(rot[:, t, :], tmp1, tmp2)

        def qk_exp(bp, t):
            """QK^T + exp for attend-block bp, t-group. Fills st_['pt']."""
            qrot = st_[(bp, "qrot")]
            kvrot = st_[(bp, "kvrot")]
            kvrot_prev = st_.get((bp - 1, "kvrot"))
            pt = [{}, {}]

            def emit_group(m_, key, ents):
                r0 = m_ * DH
                st = stps.tile([128, TB], f32, tag="stps")
                for kvr, kvcol, qoff, qw, dcol in ents:
                    nc.tensor.matmul(
                        st[:, dcol : dcol + qw],
                        lhsT=kvr[r0 : r0 + DH, t, kvcol : kvcol + 128],
                        rhs=qrot[r0 : r0 + DH, t, qoff : qoff + qw],
                        start=True,
                        stop=True,
                    )
                p = ptpool.tile([128, TB], bf16, tag="pt")
                nc.scalar.activation(p, st, EXP)
                pt[m_][key] = p

            # m0's sl=0 groups first so AV group 0's first matmuls
            # unblock earliest; c1 (needed only for sl=1) last
            if bp > 0:
                emit_group(0, "pm", [(kvrot_prev, 256 + i * 128, 0, M, i * M)
                                     for i in range(2)])
            emit_group(0, "c00", [(kvrot, 0, 0, TB, 0)])
            emit_group(0, "c01", [(kvrot, 128, 0, TB, 0)])
            if bp > 0:
                emit_group(1, "pm", [(kvrot_prev, 256 + i * 128, 0, M, i * M)
                                     for i in range(2)])
            emit_group(1, "c00", [(kvrot, 0, 0, TB, 0)])
            emit_group(1, "c01", [(kvrot, 128, 0, TB, 0)])
            emit_group(0, "c1", [(kvrot, 256 + i * 128, M, M, i * M)
                                 for i in range(2)])
            emit_group(1, "c1", [(kvrot, 256 + i * 128, M, M, i * M)
                                 for i in range(2)])
            st_[(bp, "pt")] = pt

        def av_chunks(bp, m_, sl, t):
            pt = st_[(bp, "pt")]
            kvn = st_.get((bp, "kvn"))
            kvn_prev = st_.get((bp - 1, "kvn"))
            if sl == 0:
                if bp == 0:
                    return [
                        (pt[m_]["c00"], 0, kvn, 0),
                        (pt[m_]["c01"], 0, kvn, 1),
                    ]
                return [
                    (pt[m_]["pm"], 0, kvn_prev, 2),
                    (pt[m_]["pm"], M, kvn_prev, 3),
                    (pt[m_]["c00"], 0, kvn, 0),
                    (pt[m_]["c01"], 0, kvn, 1),
                ]
            return [
                (pt[m_]["c00"], M, kvn, 0),
                (pt[m_]["c01"], M, kvn, 1),
                (pt[m_]["c1"], 0, kvn, 2),
                (pt[m_]["c1"], M, kvn, 3),
            ]

        def av_group(bp, t, sl, qc):
            """AV matmuls (natural orientation) for one (sl, qc) query chunk;
            denominators accumulate in column DH via tiny ones-matmuls."""
            av = avps.tile([128, 2, DH + 1], f32, tag="avps")
            for m_ in range(2):
                h = 2 * t + m_
                chunks = av_chunks(bp, m_, sl, t)
                for ci, (p_, pcol, kvnb, cc) in enumerate(chunks):
                    nc.tensor.matmul(
                        av[:, m_, :],
                        lhsT=p_[:, pcol + qc * 128 : pcol + qc * 128 + 128],
                        rhs=kvnb[:, cc, h, :],
                        start=(ci == 0),
                        stop=(ci == len(chunks) - 1),
                    )
            return av

        def av_norm(av, nat, j):
            """DVE: reciprocal of denoms + one broadcast multiply into nat."""
            rc = rcpool.tile([128, 2, 1], f32, tag="rc")
            nc.vector.reciprocal(rc, av[:, :, DH : DH + 1])
            nc.vector.tensor_mul(
                nat[:, j, :].rearrange("p (a b) -> p a b", a=2),
                av[:, :, 0:DH],
                rc.broadcast_to([128, 2, DH]),
            )

        def attend_av(bp, t, mid_produce=None):
            """AV + normalization + back-transpose for one (bp, t) group.
            Norms trail one AV group behind and the transposes trail the
            norms, so the PE always has an independent matmul between
            dependent pairs."""
            outT = st_[(bp, "outT")]
            nat = natpool.tile([128, 4, 128], bf16, tag="nat")
            np4 = tpps.tile([128, 4, 128], bf16, tag="tpps")
            avs = []
            for j, (sl, qc) in enumerate(((0, 0), (0, 1), (1, 0), (1, 1))):
                avs.append(av_group(bp, t, sl, qc))
                if j == 1 and mid_produce is not None:
                    mid_produce()
                if j >= 1:
                    av_norm(avs[j - 1], nat, j - 1)
                if j >= 2:
                    nc.tensor.transpose(np4[:, j - 2, :], nat[:, j - 2, :], ident)
            av_norm(avs[3], nat, 3)
            for j in (2, 3):
                nc.tensor.transpose(np4[:, j, :], nat[:, j, :], ident)
            nc.vector.tensor_copy(
                outT[:, t, :], np4.rearrange("p a b -> p (a b)")
            )

        def outproj(bp):
            outT = st_[(bp, "outT")]
            n0p = bp * TB
            if bp == 0:
                ranges = [(0, M, wo0_sb), (M, TB, wo_sb)]
            else:
                ranges = [(0, TB, wo_sb)]
            for oc in range(6):
                pps = mmps.tile([128, TB], f32, tag="mmps")
                for (a0, a1, wsb) in ranges:
                    for t in range(3):
                        nc.tensor.matmul(
                            pps[:, a0:a1],
                            lhsT=wsb[:, t, oc * 128 : (oc + 1) * 128],
                            rhs=outT[:, t, a0:a1],
                            start=(t == 0),
                            stop=(t == 2),
                        )
                prj = prpool.tile([128, TB], f32, tag="prj")
                nc.vector.tensor_copy(prj, pps)
                nc.sync.dma_start(out=outpT_r[:, oc, n0p : n0p + TB], in_=prj)
            for k in ((bp - 1, "qrot"), (bp - 1, "pt"), (bp, "pt")):
                st_.pop(k, None)

        for b in range(NB):
            produce = True
            bp = b - 1  # attend block
            last = b == NB - 1
            if b + 1 < NB:
                load_block(b + 1)
            xt, cosb, sinb = st_.pop((b, "in"))
            qrot = qrpool.tile([128, 3, TB], bf16, tag="qrot")
            kvrot = kvrpool.tile([128, 3, TB], bf16, tag="kvrot")
            rawq = rawpool.tile([128, 3, TB], bf16, tag="rawq")
            rawkv = rawpool.tile([128, 3, TB], bf16, tag="rawkv")
            kvnb = kvnpool.tile([128, 4, HPC, DH + 1], bf16, tag="kvn")
            nc.gpsimd.memset(kvnb[:, :, :, DH : DH + 1], 1.0)
            outT_b = otpool.tile([128, 3, TB], bf16, tag="outT")
            st_[(b, "qrot")] = qrot
            st_[(b, "kvrot")] = kvrot
            st_[(b, "kvn")] = kvnb
            st_[(b, "outT")] = outT_b

            for t in range(3):
                if bp >= 0:
                    qk_exp(bp, t)
                proj(wq_sb, xt, rawq, t, on_dve=True)
                shq = rot_shift(rawq, t)
                t1q, t2q = rot_muls(rawq, shq, t, cosb, sinb)
                rot_add(qrot, t, t1q, t2q)
                proj(wkv_sb, xt, rawkv, t, on_dve=True)

                def mid_kv(t=t):
                    shkv = rot_shift(rawkv, t)
                    t1k, t2k = rot_muls(rawkv, shkv, t, cosb, sinb)
                    rot_add(kvrot, t, t1k, t2k)

                if bp >= 0:
                    attend_av(bp, t, mid_kv)
                else:
                    mid_kv()
                # kv natural layout: 4 transposes into one PSUM bank,
                # one wide DVE copy off
                tp4 = tpps.tile([128, 4, 128], bf16, tag="tpps")
                for cc in range(4):
                    nc.tensor.transpose(
                        tp4[:, cc, :],
                        kvrot[:, t, cc * 128 : (cc + 1) * 128],
                        ident,
                    )
                nc.vector.tensor_copy(
                    kvnb[:, :, 2 * t : 2 * t + 2, 0:DH],
                    tp4.rearrange("p c (a b) -> p c a b", a=2),
                )
                if last and t >= 1:
                    # tighten the tail: the final block's attention runs one
                    # section behind its produce instead of a full block
                    qk_exp(b, t - 1)
                    attend_av(b, t - 1)

            if bp >= 0:
                outproj(bp)

        # epilogue: the final block's last attention section + projection
        qk_exp(NB - 1, 2)
        attend_av(NB - 1, 2)
        outproj(NB - 1)

    nc.compile()
    return nc


_CACHE = {}
TRACE = False


def kernel(x, Wq, Wkv, Wo, bo, Wo0, bo0):
    from concourse.bass_utils import run_bass_kernel_spmd
    from ml_dtypes import bfloat16

    x = np.asarray(x, dtype=np.float32)
    Wq = np.asarray(Wq, dtype=np.float32)
    Wkv = np.asarray(Wkv, dtype=np.float32)
    Wo = np.asarray(Wo, dtype=np.float32)
    bo = np.asarray(bo, dtype=np.float32)
    Wo0 = np.asarray(Wo0, dtype=np.float32)
    bo0 = np.asarray(bo0, dtype=np.float32)

    cosT, sinT, pshift = _host_tables()
    Wq_s = (Wq * SCALE).astype(np.float32)

    def bf(a):
        return np.ascontiguousarray(a).astype(bfloat16)

    xTs = [bf(x[b_].T) for b_ in range(B)]
    cosb = bf(cosT)
    sinb = bf(sinT)
    pshb = bf(pshift)
    in_maps = []
    for ci in range(NCORES):
        b_, hi = ci // 2, ci % 2
        fsl = slice(hi * FPC, (hi + 1) * FPC)
        in_maps.append(
            {
                "xT": xTs[b_],
                "wq": bf(Wq_s[:, fsl]),
                "wkv": bf(Wkv[:, fsl]),
                "wo": bf(Wo[fsl, :]),
                "wo0": bf(Wo0[fsl, :]),
                "cosT": cosb,
                "sinT": sinb,
                "pshift": pshb,
            }
        )

    if "nc" not in _CACHE:
        _CACHE["nc"] = _build_nc()
    nc = _CACHE["nc"]

    res = run_bass_kernel_spmd(
        nc, in_maps, core_ids=list(range(NCORES)), trace=TRACE
    )
    _CACHE["last"] = res
    parts = [np.asarray(r["outpT"], dtype=np.float32) for r in res.results]

    out = np.empty((B, N, DIM), dtype=np.float32)
    bias = np.empty((N, DIM), dtype=np.float32)
    bias[:M] = bo0
    bias[M:] = bo
    for b_ in range(B):
        acc = parts[2 * b_] + parts[2 * b_ + 1]      # (768, 4096)
        out[b_] = acc.T + bias
    return out


# revision 45
# speedup vs baseline: 1.0003x; 1.0003x over previous
"""LIPAR segment attention kernel for TRN2, 8 NeuronCores.

Problem (hardcoded): B=4, N=4096, DIM=768, H=12 heads, DH=64, S=16 segments
of M=256 tokens. q = x@Wq (scaled, rotary), kv = x@Wkv (rotary, shared K==V).
Segment t>=1 attends to segments [t-1, t]; segment 0 attends to itself.
Output projection Wo/bo for segments >=1, Wo0/bo0 for segment 0.

Sharding: the (b h) = 48 fused axis is split 8 ways -> 6 heads per core;
core i -> batch i//2, heads (i%2)*6 .. (i%2)*6+6. Each core computes a
partial output projection (its 384 feature rows of Wo/Wo0); the host sums
the two partial (768, 4096) results per batch and adds biases.

On-device strategy (bf16 matmul pipeline, fp32 PSUM accumulation):
  - all-bf16 inputs/weights/tables -> every matmul at 1 cycle/row.
  - software pipelining: block b's projections+rotary ("produce") are
    emitted interleaved with block b-1's attention ("attend") at t-group
    granularity so every engine always has independent queued work
    (engine streams execute strictly in order).
  - rotary in the transposed layout: rot = raw*cosT + (Pshift@raw)*sinT,
    sign baked into sinT. raw copies on ACT, sin-mul on DVE (PSUM read),
    cos-mul + add on Pool.
  - QK^T computes S^T (keys-on-partition); exp on ACT (no max subtraction),
    bf16 out.
  - AV in natural orientation (out(query, dh) = pt_chunk^T @ kvn with a
    ones column for the denominator): softmax denom is per-partition, so
    normalization is a DVE reciprocal + two per-partition tensor_scalar
    multiplies straight out of PSUM.
  - normalized (query, feat-pair) tiles are PE-transposed back to
    (feat, token); 4 transposes share one PSUM bank and are copied off
    with a single wide DVE copy (same for the kv natural-layout build).
  - output projection consumes outT directly; partial (768, 4096) fp32
    results are DMA'd out; host sums core pairs, transposes, adds biases.
  - startup DMA is need-ordered (first projection's weight chunk, block-0
    inputs, shift matrix, then the rest) so the PE starts within ~2us.

Measured (cost-model timeline, per core): 210011 ns vs 844046 ns for the
fp32 non-pipelined baseline (4.02x). HW rel l2 error 5.3e-3 (gate 2e-2).
The final block's attention is pipelined one t-section behind its produce
phase (instead of a full block) so the epilogue is a single section.
"""

import numpy as np

B, N, DIM = 4, 4096, 768
H = 12
DH = 64
S = 16
M = 256
SCALE = DH**-0.5

HPC = 6            # heads per core
FPC = HPC * DH     # 384 features per core
KC = DIM // 128    # 6 contraction chunks
NB = 8             # token blocks
TB = N // NB       # 512 tokens per block (2 segments)
NCORES = 8


def _host_tables():
    """cosT/sinT (128, N) in the 2-head-stacked transposed layout and
    the signed shift permutation (128, 128)."""
    inv_freq = 1.0 / (10000.0 ** (np.arange(0, DH, 2, dtype=np.float64) / DH))
    t = np.arange(N, dtype=np.float64)
    freqs = np.outer(inv_freq, t)            # (32, N)
    r = np.arange(128)
    fidx = r % 32
    cosT = np.cos(freqs)[fidx].astype(np.float32)            # (128, N)
    sign = np.where((r % 64) < 32, -1.0, 1.0)[:, None]
    sinT = (sign * np.sin(freqs)[fidx]).astype(np.float32)   # (128, N)
    pshift = np.zeros((128, 128), dtype=np.float32)
    for m_ in range(128):
        src = m_ + 32 if (m_ % 64) < 32 else m_ - 32
        pshift[src, m_] = 1.0
    return cosT, sinT, pshift


def _build_nc():
    import concourse.bass as bass
    import concourse.bacc as bacc
    import concourse.tile as tile
    from concourse import mybir
    from concourse.masks import make_identity
    from contextlib import ExitStack

    f32 = mybir.dt.float32
    bf16 = mybir.dt.bfloat16
    EXP = mybir.ActivationFunctionType.Exp
    MULT = mybir.AluOpType.mult

    nc = bacc.Bacc("TRN2", target_bir_lowering=False)
    xT = nc.dram_tensor("xT", [DIM, N], bf16, kind="ExternalInput")
    wq = nc.dram_tensor("wq", [DIM, FPC], bf16, kind="ExternalInput")
    wkv = nc.dram_tensor("wkv", [DIM, FPC], bf16, kind="ExternalInput")
    wo = nc.dram_tensor("wo", [FPC, DIM], bf16, kind="ExternalInput")
    wo0 = nc.dram_tensor("wo0", [FPC, DIM], bf16, kind="ExternalInput")
    cosT = nc.dram_tensor("cosT", [128, N], bf16, kind="ExternalInput")
    sinT = nc.dram_tensor("sinT", [128, N], bf16, kind="ExternalInput")
    pshift = nc.dram_tensor("pshift", [128, 128], bf16, kind="ExternalInput")
    outpT = nc.dram_tensor("outpT", [DIM, N], f32, kind="ExternalOutput")

    outpT_r = outpT.rearrange("(c p) n -> p c n", p=128)

    with tile.TileContext(nc) as tc, ExitStack() as ctx:
        consts = ctx.enter_context(tc.tile_pool(name="consts", bufs=1))
        xpool = ctx.enter_context(tc.tile_pool(name="xpool", bufs=2))
        cspool = ctx.enter_context(tc.tile_pool(name="cspool", bufs=2))
        rawpool = ctx.enter_context(tc.tile_pool(name="rawpool", bufs=3))
        tmppool = ctx.enter_context(tc.tile_pool(name="tmppool", bufs=4))
        qrpool = ctx.enter_context(tc.tile_pool(name="qrpool", bufs=2))
        kvrpool = ctx.enter_context(tc.tile_pool(name="kvrpool", bufs=3))
        kvnpool = ctx.enter_context(tc.tile_pool(name="kvnpool", bufs=3))
        ptpool = ctx.enter_context(tc.tile_pool(name="ptpool", bufs=16))
        otpool = ctx.enter_context(tc.tile_pool(name="otpool", bufs=3))
        prpool = ctx.enter_context(tc.tile_pool(name="prpool", bufs=3))
        natpool = ctx.enter_context(tc.tile_pool(name="natp", bufs=8))
        rcpool = ctx.enter_context(tc.tile_pool(name="rcp", bufs=8))

        mmps = ctx.enter_context(tc.tile_pool(name="mmps", bufs=2, space="PSUM"))
        tpps = ctx.enter_context(tc.tile_pool(name="tpps", bufs=1, space="PSUM"))
        stps = ctx.enter_context(tc.tile_pool(name="stps", bufs=3, space="PSUM"))
        avps = ctx.enter_context(tc.tile_pool(name="avps", bufs=2, space="PSUM"))

        xT_r = xT.rearrange("(c p) n -> p c n", p=128)

        # per-block live state (rotary outputs, kv natural, outT)
        st_ = {}

        def load_block(bl):
            n0 = bl * TB
            xt = xpool.tile([128, KC, TB], bf16, tag="xt")
            nc.sync.dma_start(out=xt[:, 0:3, :], in_=xT_r[:, 0:3, n0 : n0 + TB])
            nc.sync.dma_start(out=xt[:, 3:KC, :], in_=xT_r[:, 3:KC, n0 : n0 + TB])
            cosb = cspool.tile([128, TB], bf16, tag="cosb")
            nc.sync.dma_start(out=cosb, in_=cosT[:, n0 : n0 + TB])
            sinb = cspool.tile([128, TB], bf16, tag="sinb")
            nc.sync.dma_start(out=sinb, in_=sinT[:, n0 : n0 + TB])
            st_[(bl, "in")] = (xt, cosb, sinb)

        # resident constants; block-0 inputs first so the first projection
        # is not stuck behind the full weight preload
        ident = consts.tile([128, 128], bf16, tag="ident")
        make_identity(nc, ident)
        wq_r = wq.rearrange("(c p) m -> p c m", p=128)
        wkv_r = wkv.rearrange("(c p) m -> p c m", p=128)
        wq_sb = consts.tile([128, KC, FPC], bf16, tag="wq_sb")
        wkv_sb = consts.tile([128, KC, FPC], bf16, tag="wkv_sb")
        psh_sb = consts.tile([128, 128], bf16, tag="psh_sb")
        onesb = consts.tile([128, 1], bf16, tag="onesb")
        nc.vector.memset(onesb, 1.0)
        # need-ordered preload: first projection's weights, block-0 inputs,
        # the shift matrix, then the rest
        nc.sync.dma_start(out=wq_sb[:, 0:1, 0:128], in_=wq_r[:, 0:1, 0:128])
        nc.sync.dma_start(out=wq_sb[:, 1:KC, 0:128], in_=wq_r[:, 1:KC, 0:128])
        n0 = 0
        xt0 = xpool.tile([128, KC, TB], bf16, tag="xt")
        nc.sync.dma_start(out=xt0[:, 0:3, :], in_=xT_r[:, 0:3, n0 : n0 + TB])
        nc.sync.dma_start(out=xt0[:, 3:KC, :], in_=xT_r[:, 3:KC, n0 : n0 + TB])
        nc.sync.dma_start(out=psh_sb, in_=pshift[:, :])
        nc.sync.dma_start(out=wkv_sb[:, :, 0:128], in_=wkv_r[:, :, 0:128])
        cosb0 = cspool.tile([128, TB], bf16, tag="cosb")
        nc.sync.dma_start(out=cosb0, in_=cosT[:, n0 : n0 + TB])
        sinb0 = cspool.tile([128, TB], bf16, tag="sinb")
        nc.sync.dma_start(out=sinb0, in_=sinT[:, n0 : n0 + TB])
        st_[(0, "in")] = (xt0, cosb0, sinb0)
        nc.sync.dma_start(out=wq_sb[:, :, 128:FPC], in_=wq_r[:, :, 128:FPC])
        nc.sync.dma_start(out=wkv_sb[:, :, 128:FPC], in_=wkv_r[:, :, 128:FPC])
        wo_sb = consts.tile([128, 3, DIM], bf16, tag="wo_sb")
        nc.sync.dma_start(out=wo_sb, in_=wo.rearrange("(c p) m -> p c m", p=128))
        wo0_sb = consts.tile([128, 3, DIM], bf16, tag="wo0_sb")
        nc.sync.dma_start(out=wo0_sb, in_=wo0.rearrange("(c p) m -> p c m", p=128))

        def proj(wsb, xt, dst_rot_raw, t, on_dve=False):
            """projection matmuls for one 128-feature t-group + raw copy."""
            raw = dst_rot_raw
            ps = mmps.tile([128, TB], f32, tag="mmps")
            for c in range(KC):
                nc.tensor.matmul(
                    ps,
                    lhsT=wsb[:, c, t * 128 : (t + 1) * 128],
                    rhs=xt[:, c, :],
                    start=(c == 0),
                    stop=(c == KC - 1),
                )
            if on_dve:
                nc.vector.tensor_copy(raw[:, t, :], ps)
            else:
                nc.scalar.copy(raw[:, t, :], ps)

        def rot_shift(raw, t):
            shps = mmps.tile([128, TB], f32, tag="mmps")
            nc.tensor.matmul(
                shps, lhsT=psh_sb, rhs=raw[:, t, :], start=True, stop=True
            )
            return shps

        def rot_muls(raw, shps, t, cosb, sinb):
            tmp2 = tmppool.tile([128, TB], bf16, tag="tmp2")
            nc.vector.tensor_mul(tmp2, shps, sinb)
            tmp1 = tmppool.tile([128, TB], bf16, tag="tmp1")
            nc.gpsimd.tensor_mul(tmp1, raw[:, t, :], cosb)
            return tmp1, tmp2

        def rot_add(rot, t, tmp1, tmp2):
            nc.gpsimd.tensor_add(rot[:, t, :], tmp1, tmp2)

        def qk_exp(bp, t):
            """QK^T + exp for attend-block bp, t-group. Fills st_['pt']."""
            qrot = st_[(bp, "qrot")]
            kvrot = st_[(bp, "kvrot")]
            kvrot_prev = st_.get((bp - 1, "kvrot"))
            pt = [{}, {}]

            def emit_group(m_, key, ents):
                r0 = m_ * DH
                st = stps.tile([128, TB], f32, tag="stps")
                for kvr, kvcol, qoff, qw, dcol in ents:
                    nc.tensor.matmul(
                        st[:, dcol : dcol + qw],
                        lhsT=kvr[r0 : r0 + DH, t, kvcol : kvcol + 128],
                        rhs=qrot[r0 : r0 + DH, t, qoff : qoff + qw],
                        start=True,
                        stop=True,
                    )
                p = ptpool.tile([128, TB], bf16, tag="pt")
                nc.scalar.activation(p, st, EXP)
                pt[m_][key] = p

            # m0's sl=0 groups first so AV group 0's first matmuls
            # unblock earliest; c1 (needed only for sl=1) last
            if bp > 0:
                emit_group(0, "pm", [(kvrot_prev, 256 + i * 128, 0, M, i * M)
                                     for i in range(2)])
            emit_group(0, "c00", [(kvrot, 0, 0, TB, 0)])
            emit_group(0, "c01", [(kvrot, 128, 0, TB, 0)])
            if bp > 0:
                emit_group(1, "pm", [(kvrot_prev, 256 + i * 128, 0, M, i * M)
                                     for i in range(2)])
            emit_group(1, "c00", [(kvrot, 0, 0, TB, 0)])
            emit_group(1, "c01", [(kvrot, 128, 0, TB, 0)])
            emit_group(0, "c1", [(kvrot, 256 + i * 128, M, M, i * M)
                                 for i in range(2)])
            emit_group(1, "c1", [(kvrot, 256 + i * 128, M, M, i * M)
                                 for i in range(2)])
            st_[(bp, "pt")] = pt

        def av_chunks(bp, m_, sl, t):
            pt = st_[(bp, "pt")]
            kvn = st_.get((bp, "kvn"))
            kvn_prev = st_.get((bp - 1, "kvn"))
            if sl == 0:
                if bp == 0:
                    return [
                        (pt[m_]["c00"], 0, kvn, 0),
                        (pt[m_]["c01"], 0, kvn, 1),
                    ]
                return [
                    (pt[m_]["pm"], 0, kvn_prev, 2),
                    (pt[m_]["pm"], M, kvn_prev, 3),
                    (pt[m_]["c00"], 0, kvn, 0),
                    (pt[m_]["c01"], 0, kvn, 1),
                ]
            return [
                (pt[m_]["c00"], M, kvn, 0),
                (pt[m_]["c01"], M, kvn, 1),
                (pt[m_]["c1"], 0, kvn, 2),
                (pt[m_]["c1"], M, kvn, 3),
            ]

        def av_group(bp, t, sl, qc):
            """AV matmuls (natural orientation) for one (sl, qc) query chunk;
            denominators accumulate in column DH via tiny ones-matmuls."""
            av = avps.tile([128, 2, DH + 1], f32, tag="avps")
            for m_ in range(2):
                h = 2 * t + m_
                chunks = av_chunks(bp, m_, sl, t)
                for ci, (p_, pcol, kvnb, cc) in enumerate(chunks):
                    nc.tensor.matmul(
                        av[:, m_, :],
                        lhsT=p_[:, pcol + qc * 128 : pcol + qc * 128 + 128],
                        rhs=kvnb[:, cc, h, :],
                        start=(ci == 0),
                        stop=(ci == len(chunks) - 1),
                    )
            return av

        def av_norm(av, nat, j):
            """DVE: reciprocal of denoms + one broadcast multiply into nat."""
            rc = rcpool.tile([128, 2, 1], f32, tag="rc")
            nc.vector.reciprocal(rc, av[:, :, DH : DH + 1])
            nc.vector.tensor_mul(
                nat[:, j, :].rearrange("p (a b) -> p a b", a=2),
                av[:, :, 0:DH],
                rc.broadcast_to([128, 2, DH]),
            )

        def attend_av(bp, t, mid_produce=None):
            """AV + normalization + back-transpose for one (bp, t) group.
            Norms trail one AV group behind and the transposes trail the
            norms, so the PE always has an independent matmul between
            dependent pairs."""
            outT = st_[(bp, "outT")]
            nat = natpool.tile([128, 4, 128], bf16, tag="nat")
            np4 = tpps.tile([128, 4, 128], bf16, tag="tpps")
            avs = []
            for j, (sl, qc) in enumerate(((0, 0), (0, 1), (1, 0), (1, 1))):
                avs.append(av_group(bp, t, sl, qc))
                if j == 1 and mid_produce is not None:
                    mid_produce()
                if j >= 1:
                    av_norm(avs[j - 1], nat, j - 1)
                if j >= 2:
                    nc.tensor.transpose(np4[:, j - 2, :], nat[:, j - 2, :], ident)
            av_norm(avs[3], nat, 3)
            for j in (2, 3):
                nc.tensor.transpose(np4[:, j, :], nat[:, j, :], ident)
            nc.vector.tensor_copy(
                outT[:, t, :], np4.rearrange("p a b -> p (a b)")
            )

        def outproj(bp):
            outT = st_[(bp, "outT")]
            n0p = bp * TB
            if bp == 0:
                ranges = [(0, M, wo0_sb), (M, TB, wo_sb)]
            else:
                ranges = [(0, TB, wo_sb)]
            for oc in range(6):
                pps = mmps.tile([128, TB], f32, tag="mmps")
                for (a0, a1, wsb) in ranges:
                    for t in range(3):
                        nc.tensor.matmul(
                            pps[:, a0:a1],
                            lhsT=wsb[:, t, oc * 128 : (oc + 1) * 128],
                            rhs=outT[:, t, a0:a1],
                            start=(t == 0),
                            stop=(t == 2),
                        )
                prj = prpool.tile([128, TB], f32, tag="prj")
                nc.vector.tensor_copy(prj, pps)
                nc.sync.dma_start(out=outpT_r[:, oc, n0p : n0p + TB], in_=prj)
            for k in ((bp - 1, "qrot"), (bp - 1, "pt"), (bp, "pt")):
                st_.pop(k, None)

        for b in range(NB):
            produce = True
            bp = b - 1  # attend block
            last = b == NB - 1
            if b + 1 < NB:
                load_block(b + 1)
            xt, cosb, sinb = st_.pop((b, "in"))
            qrot = qrpool.tile([128, 3, TB], bf16, tag="qrot")
            kvrot = kvrpool.tile([128, 3, TB], bf16, tag="kvrot")
            rawq = rawpool.tile([128, 3, TB], bf16, tag="rawq")
            rawkv = rawpool.tile([128, 3, TB], bf16, tag="rawkv")
            kvnb = kvnpool.tile([128, 4, HPC, DH + 1], bf16, tag="kvn")
            nc.gpsimd.memset(kvnb[:, :, :, DH : DH + 1], 1.0)
            outT_b = otpool.tile([128, 3, TB], bf16, tag="outT")
            st_[(b, "qrot")] = qrot
            st_[(b, "kvrot")] = kvrot
            st_[(b, "kvn")] = kvnb
            st_[(b, "outT")] = outT_b

            for t in range(3):
                if bp >= 0:
                    qk_exp(bp, t)
                proj(wq_sb, xt, rawq, t, on_dve=True)
                shq = rot_shift(rawq, t)
                t1q, t2q = rot_muls(rawq, shq, t, cosb, sinb)
                rot_add(qrot, t, t1q, t2q)
                proj(wkv_sb, xt, rawkv, t, on_dve=True)

                def mid_kv(t=t):
                    shkv = rot_shift(rawkv, t)
                    t1k, t2k = rot_muls(rawkv, shkv, t, cosb, sinb)
                    rot_add(kvrot, t, t1k, t2k)

                if bp >= 0:
                    attend_av(bp, t, mid_kv)
                else:
                    mid_kv()
                # kv natural layout: 4 transposes into one PSUM bank,
                # one wide DVE copy off
                tp4 = tpps.tile([128, 4, 128], bf16, tag="tpps")
                for cc in range(4):
                    nc.tensor.transpose(
                        tp4[:, cc, :],
                        kvrot[:, t, cc * 128 : (cc + 1) * 128],
                        ident,
                    )
                nc.vector.tensor_copy(
                    kvnb[:, :, 2 * t : 2 * t + 2, 0:DH],
                    tp4.rearrange("p c (a b) -> p c a b", a=2),
                )
                if last and t >= 1:
                    # tighten the tail: the final block's attention runs one
                    # section behind its produce instead of a full block
                    qk_exp(b, t - 1)
                    attend_av(b, t - 1)

            if bp >= 0:
                outproj(bp)

        # epilogue: the final block's last attention section + projection
        qk_exp(NB - 1, 2)
        attend_av(NB - 1, 2)
        outproj(NB - 1)

    nc.compile()
    return nc


_CACHE = {}
TRACE = False


def kernel(x, Wq, Wkv, Wo, bo, Wo0, bo0):
    from concourse.bass_utils import run_bass_kernel_spmd
    from ml_dtypes import bfloat16

    x = np.asarray(x, dtype=np.float32)
    Wq = np.asarray(Wq, dtype=np.float32)
    Wkv = np.asarray(Wkv, dtype=np.float32)
    Wo = np.asarray(Wo, dtype=np.float32)
    bo = np.asarray(bo, dtype=np.float32)
    Wo0 = np.asarray(Wo0, dtype=np.float32)
    bo0 = np.asarray(bo0, dtype=np.float32)

    cosT, sinT, pshift = _host_tables()
    Wq_s = (Wq * SCALE).astype(np.float32)

    def bf(a):
        return np.ascontiguousarray(a).astype(bfloat16)

    xTs = [bf(x[b_].T) for b_ in range(B)]
    cosb = bf(cosT)
    sinb = bf(sinT)
    pshb = bf(pshift)
    in_maps = []
    for ci in range(NCORES):
        b_, hi = ci // 2, ci % 2
        fsl = slice(hi * FPC, (hi + 1) * FPC)
        in_maps.append(
            {
                "xT": xTs[b_],
                "wq": bf(Wq_s[:, fsl]),
                "wkv": bf(Wkv[:, fsl]),
                "wo": bf(Wo[fsl, :]),
                "wo0": bf(Wo0[fsl, :]),
                "cosT": cosb,
                "sinT": sinb,
                "pshift": pshb,
            }
        )

    if "nc" not in _CACHE:
        _CACHE["nc"] = _build_nc()
    nc = _CACHE["nc"]

    res = run_bass_kernel_spmd(
        nc, in_maps, core_ids=list(range(NCORES)), trace=TRACE
    )
    _CACHE["last"] = res
    parts = [np.asarray(r["outpT"], dtype=np.float32) for r in res.results]

    out = np.empty((B, N, DIM), dtype=np.float32)
    bias = np.empty((N, DIM), dtype=np.float32)
    bias[:M] = bo0
    bias[M:] = bo
    for b_ in range(B):
        acc = parts[2 * b_] + parts[2 * b_ + 1]      # (768, 4096)
        out[b_] = acc.T + bias
    return out
